# revision 23
# baseline (speedup 1.0000x reference)
"""Bot-detection transformer forward pass on 8 Trainium2 NeuronCores.

Strategy: data-parallel over batch (B=8 -> 1 sequence per core, no
collectives).  On each core the residual stream is kept transposed
(feature-major, xT: [768 x 1024] as 6 partition-tiles) so that every
projection runs with the weight block as the stationary matmul operand
and activations as the moving operand -- no activation transposes
anywhere in the layer loop.

Attention is computed as scores^T (keys on partitions).  Softmax needs
no max subtraction (scores are bounded); the key-padding mask folds
into v as a per-partition scale, and the softmax denominator falls out
of the att.v matmul as a 65th stationary column.  All matmuls run in
float32r (fp32 data, full PE rate at N>=256).
"""

import math

import numpy as np

B, S, D, H, L, V, C = 8, 1024, 768, 12, 6, 32000, 2
HD, DF, MAXPOS = 64, 3072, 2048
P = 128
KT = D // P    # 6 feature tiles
NT = S // P    # 8 token tiles
FT = DF // P   # 24 ff tiles
NQ = 2         # query halves of 512
QW = S // NQ   # 512
EPS = 1e-5
N_CORES = 8

_CACHE = {}


def _build_nc(n_layers=L):
    import concourse.bass as bass
    import concourse.tile as tile
    from concourse import bacc, mybir
    from concourse.bass import ds, ts
    from concourse.masks import make_identity
    from contextlib import ExitStack

    f32 = mybir.dt.float32
    bf16 = mybir.dt.bfloat16
    f32r = mybir.dt.float32r
    i32 = mybir.dt.int32
    AF = mybir.ActivationFunctionType
    OP = mybir.AluOpType

    nc = bacc.Bacc("TRN2", target_bir_lowering=False, debug=False)

    # ---------------- DRAM I/O ----------------
    d_ids = nc.dram_tensor("ids", [P, NT], i32, kind="ExternalInput")
    d_gm = nc.dram_tensor("gmask", [P, NT], f32, kind="ExternalInput")
    d_emb = nc.dram_tensor("emb", [V, D], f32, kind="ExternalInput")
    d_posT = nc.dram_tensor("posT", [D, S], f32, kind="ExternalInput")
    d_qkw = nc.dram_tensor("qkw", [L, 12, D, P], bf16, kind="ExternalInput")
    d_vw = nc.dram_tensor("vw", [L, KT, P, D], bf16, kind="ExternalInput")
    d_qkb = nc.dram_tensor("qkb", [L, P, 12], f32, kind="ExternalInput")
    d_vb = nc.dram_tensor("vb", [L, D], f32, kind="ExternalInput")
    d_ow = nc.dram_tensor("ow", [L, KT, D, P], bf16, kind="ExternalInput")
    d_ob = nc.dram_tensor("ob", [L, D], f32, kind="ExternalInput")
    d_n1s = nc.dram_tensor("n1s", [L, P, KT], f32, kind="ExternalInput")
    d_n1b = nc.dram_tensor("n1b", [L, P, KT], f32, kind="ExternalInput")
    d_n2s = nc.dram_tensor("n2s", [L, P, KT], f32, kind="ExternalInput")
    d_n2b = nc.dram_tensor("n2b", [L, P, KT], f32, kind="ExternalInput")
    d_f1w = nc.dram_tensor("f1w", [L, FT, D, P], bf16, kind="ExternalInput")
    d_f1b = nc.dram_tensor("f1b", [L, P, FT], f32, kind="ExternalInput")
    d_f2w = nc.dram_tensor("f2w", [L, KT, DF, P], bf16, kind="ExternalInput")
    d_f2b = nc.dram_tensor("f2b", [L, D], f32, kind="ExternalInput")
    d_hls = nc.dram_tensor("hls", [P, KT], f32, kind="ExternalInput")
    d_hlb = nc.dram_tensor("hlb", [P, KT], f32, kind="ExternalInput")
    d_cw = nc.dram_tensor("cw", [D, C], f32, kind="ExternalInput")
    d_cb = nc.dram_tensor("cb", [1, C], f32, kind="ExternalInput")
    d_out = nc.dram_tensor("out", [1, C], f32, kind="ExternalOutput")

    with tile.TileContext(nc) as tc, ExitStack() as ctx:
        # ---------------- pools ----------------
        state = ctx.enter_context(tc.tile_pool(name="state", bufs=1))
        consts = ctx.enter_context(tc.tile_pool(name="consts", bufs=1))
        b24 = ctx.enter_context(tc.tile_pool(name="b24", bufs=1))
        p48 = ctx.enter_context(tc.tile_pool(name="p48", bufs=1))
        vpool = ctx.enter_context(tc.tile_pool(name="vpool", bufs=1))
        vwpool = ctx.enter_context(tc.tile_pool(name="vwpool", bufs=1))
        w6 = ctx.enter_context(tc.tile_pool(name="w6", bufs=3))
        wff2 = ctx.enter_context(tc.tile_pool(name="wff2", bufs=3))
        epool = ctx.enter_context(tc.tile_pool(name="epool", bufs=3))
        tmp = ctx.enter_context(tc.tile_pool(name="tmp", bufs=3))
        zpool = ctx.enter_context(tc.tile_pool(name="zpool", bufs=1))
        rows = ctx.enter_context(tc.tile_pool(name="rows", bufs=1))
        srows = ctx.enter_context(tc.tile_pool(name="srows", bufs=2))
        srows1 = ctx.enter_context(tc.tile_pool(name="srows1", bufs=1))
        params = ctx.enter_context(tc.tile_pool(name="params", bufs=2))
        # PSUM: pscore(2x2 banks) + pmm(2) + patt(2) = 8 banks
        pscore = ctx.enter_context(tc.tile_pool(name="pscore", bufs=2, space="PSUM"))
        pmm = ctx.enter_context(tc.tile_pool(name="pmm", bufs=2, space="PSUM"))
        patt = ctx.enter_context(tc.tile_pool(name="patt", bufs=2, space="PSUM"))

        # ---------------- constants ----------------
        xT = state.tile([P, KT, S], f32r, tag="xT")
        ones_f32 = consts.tile([P, 1], f32, tag="ones_f32")
        nc.vector.memset(ones_f32[:, :], 1.0)
        ones_rf32 = consts.tile([1, QW], f32, tag="ones_rf32")
        nc.vector.memset(ones_rf32[:, :], 1.0)
        ones_col = consts.tile([P, 1], f32r, tag="ones_col")
        nc.vector.tensor_copy(out=ones_col[:, :], in_=ones_f32[:, :])
        ones_row = consts.tile([1, QW], f32r, tag="ones_row")
        nc.vector.tensor_copy(out=ones_row[:, :], in_=ones_rf32[:, :])
        ident = consts.tile([P, P], f32, tag="ident")
        make_identity(nc, ident[:, :])
        eps_sb = consts.tile([1, 1], f32, tag="eps")
        nc.vector.memset(eps_sb[:, :], EPS)
        ids_sb = consts.tile([P, NT], i32, tag="ids")
        nc.sync.dma_start(out=ids_sb[:, :], in_=d_ids[:, :])
        gcol = consts.tile([P, NT], f32, tag="gcol")
        nc.sync.dma_start(out=gcol[:, :], in_=d_gm[:, :])
        hls_sb = consts.tile([P, KT], f32, tag="hls")
        nc.sync.dma_start(out=hls_sb[:, :], in_=d_hls[:, :])
        hlb_sb = consts.tile([P, KT], f32, tag="hlb")
        nc.sync.dma_start(out=hlb_sb[:, :], in_=d_hlb[:, :])
        cw_sb = consts.tile([P, KT, C], f32r, tag="cw")
        nc.sync.dma_start(
            out=cw_sb[:, :, :],
            in_=d_cw.rearrange("(j p) c -> p j c", p=P).bitcast(f32r),
        )
        cb_sb = consts.tile([1, C], f32r, tag="cb")
        nc.sync.dma_start(out=cb_sb[:, :], in_=d_cb[:, :].bitcast(f32r))

        # ---------------- embedding ----------------
        posT_sb = b24.tile([P, KT, S], f32, tag="b24")
        nc.sync.dma_start(
            out=posT_sb[:, :, :], in_=d_posT.rearrange("(j p) s -> p j s", p=P)
        )
        for t in range(NT):
            embt = tmp.tile([P, D], f32, tag="tmp")
            nc.gpsimd.indirect_dma_start(
                out=embt[:, :],
                out_offset=None,
                in_=d_emb[:, :],
                in_offset=bass.IndirectOffsetOnAxis(ap=ids_sb[:, t : t + 1], axis=0),
            )
            # transpose 6 (128,128) blocks; pack j=0..3 and j=4..5 into psum
            ptr0 = pmm.tile([P, QW], f32, tag="pmm")
            for j in range(4):
                nc.tensor.transpose(
                    out=ptr0[:, j * P : (j + 1) * P],
                    in_=embt[:, j * P : (j + 1) * P],
                    identity=ident[:, :],
                )
            ptr1 = pmm.tile([P, QW], f32, tag="pmm")
            for j in range(2):
                nc.tensor.transpose(
                    out=ptr1[:, j * P : (j + 1) * P],
                    in_=embt[:, (4 + j) * P : (5 + j) * P],
                    identity=ident[:, :],
                )
            nc.vector.tensor_tensor(
                out=xT[:, 0:4, t * P : (t + 1) * P],
                in0=ptr0[:, :].rearrange("p (a b) -> p a b", b=P),
                in1=posT_sb[:, 0:4, t * P : (t + 1) * P],
                op=OP.add,
            )
            nc.vector.tensor_tensor(
                out=xT[:, 4:6, t * P : (t + 1) * P],
                in0=ptr1[:, 0 : 2 * P].rearrange("p (a b) -> p a b", b=P),
                in1=posT_sb[:, 4:6, t * P : (t + 1) * P],
                op=OP.add,
            )

        # ---------------- helpers ----------------
        def layer_norm(src, dst, s_sb, b_sb):
            """src (P,KT,S) f32r -> dst = layernorm over features, *s + b."""
            for n in range(NQ):
                nsl = ds(n * QW, QW)
                psum = pmm.tile([1, QW], f32, tag="pmm")
                psq = pmm.tile([1, QW], f32, tag="pmm")
                for j in range(KT):
                    xsq = tmp.tile([P, QW], f32r, tag="tmp")
                    nc.vector.tensor_tensor(
                        out=xsq[:, :], in0=src[:, j, nsl], in1=src[:, j, nsl],
                        op=OP.mult,
                    )
                    nc.tensor.matmul(
                        psum[:, :], ones_col[:, :], src[:, j, nsl],
                        start=(j == 0), stop=(j == KT - 1),
                    )
                    nc.tensor.matmul(
                        psq[:, :], ones_col[:, :], xsq[:, :],
                        start=(j == 0), stop=(j == KT - 1),
                    )
                mean = srows1.tile([1, QW], f32r, tag="mean")
                nc.vector.tensor_scalar(
                    out=mean[:, :], in0=psum[:, :], scalar1=1.0 / D, scalar2=None,
                    op0=OP.mult,
                )
                msq = srows.tile([1, QW], f32, tag="srow")
                nc.vector.tensor_scalar(
                    out=msq[:, :], in0=psq[:, :], scalar1=1.0 / D, scalar2=None,
                    op0=OP.mult,
                )
                var = srows.tile([1, QW], f32, tag="srow")
                # var = msq - mean*mean: first -mean^2 = (mean * -1) * mean
                nc.vector.scalar_tensor_tensor(
                    out=var[:, :], in0=mean[:, :], scalar=-1.0, in1=mean[:, :],
                    op0=OP.mult, op1=OP.mult,
                )
                nc.vector.tensor_tensor(
                    out=var[:, :], in0=var[:, :], in1=msq[:, :], op=OP.add,
                )
                lnv = srows.tile([1, QW], f32, tag="srow")
                nc.scalar.activation(lnv[:, :], var[:, :], AF.Ln, bias=eps_sb[:, :])
                rstd = srows1.tile([1, QW], f32r, tag="rstd")
                nc.scalar.activation(rstd[:, :], lnv[:, :], AF.Exp, scale=-0.5)
                mb = pmm.tile([P, QW], f32, tag="pmm")
                nc.tensor.matmul(
                    mb[:, :], ones_row[:, 0:P], mean[:, :], start=True, stop=True
                )
                rb = pmm.tile([P, QW], f32, tag="pmm")
                nc.tensor.matmul(
                    rb[:, :], ones_row[:, 0:P], rstd[:, :], start=True, stop=True
                )
                for j in range(KT):
                    t1 = tmp.tile([P, QW], f32, tag="tmp")
                    nc.vector.tensor_tensor(
                        out=t1[:, :], in0=src[:, j, nsl], in1=mb[:, :], op=OP.subtract
                    )
                    t2 = tmp.tile([P, QW], f32, tag="tmp")
                    nc.vector.tensor_tensor(
                        out=t2[:, :], in0=t1[:, :], in1=rb[:, :], op=OP.mult
                    )
                    nc.vector.tensor_scalar(
                        out=dst[:, j, nsl], in0=t2[:, :],
                        scalar1=s_sb[:, j : j + 1], scalar2=b_sb[:, j : j + 1],
                        op0=OP.mult, op1=OP.add,
                    )

        # hmm: var = msq - mean^2 done as (mean*-1*mean) + msq above

        # ---------------- layers ----------------
        for l in range(n_layers):
            n1s_sb = params.tile([P, KT], f32, tag="n1s")
            nc.sync.dma_start(out=n1s_sb[:, :], in_=d_n1s[l])
            n1b_sb = params.tile([P, KT], f32, tag="n1b")
            nc.sync.dma_start(out=n1b_sb[:, :], in_=d_n1b[l])

            hT = b24.tile([P, KT, S], bf16, tag="b24")
            layer_norm(xT, hT, n1s_sb, n1b_sb)

            # ---- q,k projections (feature-major output) ----
            qkb_sb = params.tile([P, 12], f32, tag="qkb")
            nc.sync.dma_start(out=qkb_sb[:, :], in_=d_qkb[l])
            qk_sb = p48.tile([P, 12, S], bf16, tag="p48")
            for m in range(12):
                wt = w6.tile([P, KT, P], bf16, tag="w6")
                nc.sync.dma_start(
                    out=wt[:, :, :],
                    in_=d_qkw[l, m].rearrange("(j p) c -> p j c", p=P),
                )
                for n in range(NQ):
                    acc = pmm.tile([P, QW], f32, tag="pmm")
                    for j in range(KT):
                        nc.tensor.matmul(
                            acc[:, :], wt[:, j, :], hT[:, j, ds(n * QW, QW)],
                            start=(j == 0), stop=(j == KT - 1),
                        )
                    nc.vector.tensor_scalar(
                        out=qk_sb[:, m, ds(n * QW, QW)], in0=acc[:, :],
                        scalar1=qkb_sb[:, m : m + 1], scalar2=None, op0=OP.add,
                    )

            # ---- v projection (token-major output) + mask fold ----
            vw_sb = vwpool.tile([P, KT, D], bf16, tag="vw")
            nc.sync.dma_start(
                out=vw_sb[:, :, :],
                in_=d_vw[l].rearrange("j p d -> p j d"),
            )
            vb_row = rows.tile([1, D], f32r, tag="brow")
            nc.sync.dma_start(out=vb_row[:, :], in_=d_vb[l : l + 1, :].bitcast(f32r))
            v_sb = vpool.tile([P, NT, H, 66], bf16, tag="v")
            nc.vector.memset(v_sb[:, :, :, 65:66], 0.0)
            for t in range(NT):
                for c0, cn in ((0, QW), (QW, D - QW)):
                    acc = pmm.tile([P, QW], f32, tag="pmm")
                    nc.tensor.matmul(
                        acc[:, 0:cn], ones_row[0:1, 0:P], vb_row[:, c0 : c0 + cn],
                        start=True, stop=False,
                    )
                    for j in range(KT):
                        nc.tensor.matmul(
                            acc[:, 0:cn], hT[:, j, ts(t, P)],
                            vw_sb[:, j, c0 : c0 + cn],
                            start=False, stop=(j == KT - 1),
                        )
                    nc.vector.tensor_scalar(
                        out=v_sb[:, t, c0 // HD : (c0 + cn) // HD, 0:HD],
                        in0=acc[:, 0:cn].rearrange("p (h d) -> p h d", d=HD),
                        scalar1=gcol[:, t : t + 1], scalar2=None, op0=OP.mult,
                    )
                nc.vector.tensor_copy(
                    out=v_sb[:, t, :, HD : HD + 1],
                    in_=gcol[:, t : t + 1].to_broadcast([P, H, 1]),
                )

            # ---- attention ----
            # head pairs (2hp, 2hp+1) live in PE row-groups 0:64 / 64:128 of
            # the same qk m-tile: their score matmuls use disjoint row groups
            # and run concurrently; one wide exp covers both.
            attT = b24.tile([P, KT, S], bf16, tag="b24")
            for hp in range(H // 2):
                hA, hB = 2 * hp, 2 * hp + 1
                for n in range(NQ):
                    nsl = ds(n * QW, QW)
                    patA = patt.tile([HD + 1, QW], f32, tag="patt")
                    patB = patt.tile([HD + 1, QW], f32, tag="patt")
                    for kt in range(NT):
                        ps = pscore.tile([P, 2 * QW], f32, tag="ps")
                        nc.tensor.matmul(
                            ps[:, 0:QW],
                            qk_sb[0:HD, 6 + hp, ts(kt, P)],
                            qk_sb[0:HD, hp, nsl],
                            start=True, stop=True,
                        )
                        nc.tensor.matmul(
                            ps[:, QW : 2 * QW],
                            qk_sb[HD:P, 6 + hp, ts(kt, P)],
                            qk_sb[HD:P, hp, nsl],
                            start=True, stop=True,
                        )
                        e = epool.tile([P, 2 * QW], bf16, tag="e")
                        nc.scalar.activation(e[:, :], ps[:, :], AF.Exp)
                        nc.tensor.matmul(
                            patA[:, :], v_sb[:, kt, hA, 0:65], e[:, 0:QW],
                            start=(kt == 0), stop=(kt == NT - 1),
                        )
                        nc.tensor.matmul(
                            patB[:, :], v_sb[:, kt, hB, 0:65], e[:, QW : 2 * QW],
                            start=(kt == 0), stop=(kt == NT - 1),
                        )
                    for pat, po in ((patA, 0), (patB, HD)):
                        zinv = srows.tile([1, QW], f32r, tag="zinv")
                        with nc.allow_low_precision(
                            reason="fp32r feed for PE broadcast"
                        ):
                            nc.vector.reciprocal(zinv[:, :], pat[HD : HD + 1, :])
                        zb = pmm.tile([P, QW], f32, tag="pmm")
                        nc.tensor.matmul(
                            zb[0:HD, :], ones_row[0:1, 0:HD], zinv[:, :],
                            start=True, stop=True,
                        )
                        zbs = zpool.tile([HD, QW], f32, tag="zbs")
                        nc.vector.tensor_copy(out=zbs[:, :], in_=zb[0:HD, :])
                        nc.vector.tensor_tensor(
                            out=attT[po : po + HD, hp, nsl],
                            in0=pat[0:HD, :], in1=zbs[:, :], op=OP.mult,
                        )

            # ---- output projection + residual ----
            ob_row = rows.tile([1, D], f32r, tag="brow")
            nc.sync.dma_start(out=ob_row[:, :], in_=d_ob[l : l + 1, :].bitcast(f32r))
            for m in range(KT):
                wt = w6.tile([P, KT, P], bf16, tag="w6")
                nc.sync.dma_start(
                    out=wt[:, :, :],
                    in_=d_ow[l, m].rearrange("(j p) c -> p j c", p=P),
                )
                for n in range(NQ):
                    acc = pmm.tile([P, QW], f32, tag="pmm")
                    nc.tensor.matmul(
                        acc[:, :], ob_row[:, m * P : (m + 1) * P], ones_row[:, :],
                        start=True, stop=False,
                    )
                    for j in range(KT):
                        nc.tensor.matmul(
                            acc[:, :], wt[:, j, :], attT[:, j, ds(n * QW, QW)],
                            start=False, stop=(j == KT - 1),
                        )
                    nc.vector.tensor_tensor(
                        out=xT[:, m, ds(n * QW, QW)], in0=xT[:, m, ds(n * QW, QW)],
                        in1=acc[:, :], op=OP.add,
                    )

            # ---- LN2 + feed-forward ----
            n2s_sb = params.tile([P, KT], f32, tag="n2s")
            nc.sync.dma_start(out=n2s_sb[:, :], in_=d_n2s[l])
            n2b_sb = params.tile([P, KT], f32, tag="n2b")
            nc.sync.dma_start(out=n2b_sb[:, :], in_=d_n2b[l])
            h2 = b24.tile([P, KT, S], bf16, tag="b24")
            layer_norm(xT, h2, n2s_sb, n2b_sb)

            f1b_sb = params.tile([P, FT], f32, tag="f1b")
            nc.sync.dma_start(out=f1b_sb[:, :], in_=d_f1b[l])
            f2b_row = rows.tile([1, D], f32r, tag="brow")
            nc.sync.dma_start(out=f2b_row[:, :], in_=d_f2b[l : l + 1, :].bitcast(f32r))
            f_sb = p48.tile([P, FT, S], bf16, tag="p48")
            for m in range(FT):
                wt = w6.tile([P, KT, P], bf16, tag="w6")
                nc.sync.dma_start(
                    out=wt[:, :, :],
                    in_=d_f1w[l, m].rearrange("(j p) c -> p j c", p=P),
                )
                for n in range(NQ):
                    nsl = ds(n * QW, QW)
                    acc = pmm.tile([P, QW], f32, tag="pmm")
                    for j in range(KT):
                        nc.tensor.matmul(
                            acc[:, :], wt[:, j, :], h2[:, j, nsl],
                            start=(j == 0), stop=(j == KT - 1),
                        )
                    nc.vector.tensor_scalar(
                        out=f_sb[:, m, nsl], in0=acc[:, :],
                        scalar1=f1b_sb[:, m : m + 1], scalar2=0.0,
                        op0=OP.add, op1=OP.max,
                    )
            for m in range(KT):
                accs = []
                for n in range(NQ):
                    acc = pmm.tile([P, QW], f32, tag="pmm")
                    nc.tensor.matmul(
                        acc[:, :], f2b_row[:, m * P : (m + 1) * P], ones_row[:, :],
                        start=True, stop=False,
                    )
                    accs.append(acc)
                for half in range(2):
                    w2 = wff2.tile([P, FT // 2, P], bf16, tag="wff2")
                    nc.sync.dma_start(
                        out=w2[:, :, :],
                        in_=d_f2w[l, m, half * (DF // 2) : (half + 1) * (DF // 2)]
                        .rearrange("(j p) c -> p j c", p=P),
                    )
                    for jj in range(FT // 2):
                        j_abs = half * (FT // 2) + jj
                        for n in range(NQ):
                            nc.tensor.matmul(
                                accs[n][:, :], w2[:, jj, :],
                                f_sb[:, j_abs, ds(n * QW, QW)],
                                start=False, stop=(j_abs == FT - 1),
                            )
                for n in range(NQ):
                    nsl = ds(n * QW, QW)
                    nc.vector.tensor_tensor(
                        out=xT[:, m, nsl], in0=xT[:, m, nsl], in1=accs[n][:, :],
                        op=OP.add,
                    )

        # ---------------- CLS head ----------------
        # computed on token columns 0:2 (width 2 keeps fp32r matmul ISA
        # restrictions satisfied); only column 0 is used for the output.
        col2 = xT[:, :, 0:2]  # (P, KT, 2)
        xsqc = consts.tile([P, KT, 2], f32r, tag="xsqc")
        nc.vector.tensor_tensor(out=xsqc[:, :, :], in0=col2, in1=col2, op=OP.mult)
        pss = pmm.tile([1, QW], f32, tag="pmm")
        for j in range(KT):
            nc.tensor.matmul(
                pss[:, 0:2], ones_col[:, :], xT[:, j, 0:2],
                start=(j == 0), stop=(j == KT - 1),
            )
        for j in range(KT):
            nc.tensor.matmul(
                pss[:, 2:4], ones_col[:, :], xsqc[:, j, :],
                start=(j == 0), stop=(j == KT - 1),
            )
        hmean = srows1.tile([1, QW], f32r, tag="mean")
        nc.vector.tensor_scalar(
            out=hmean[:, 0:2], in0=pss[:, 0:2], scalar1=1.0 / D, scalar2=None,
            op0=OP.mult,
        )
        hmsq = srows.tile([1, QW], f32, tag="srow")
        nc.vector.tensor_scalar(
            out=hmsq[:, 0:2], in0=pss[:, 2:4], scalar1=1.0 / D, scalar2=None,
            op0=OP.mult,
        )
        hvar = srows.tile([1, QW], f32, tag="srow")
        nc.vector.scalar_tensor_tensor(
            out=hvar[:, 0:2], in0=hmean[:, 0:2], scalar=-1.0, in1=hmean[:, 0:2],
            op0=OP.mult, op1=OP.mult,
        )
        nc.vector.tensor_tensor(
            out=hvar[:, 0:2], in0=hvar[:, 0:2], in1=hmsq[:, 0:2], op=OP.add
        )
        hlnv = srows.tile([1, QW], f32, tag="srow")
        nc.scalar.activation(hlnv[:, 0:2], hvar[:, 0:2], AF.Ln, bias=eps_sb[:, :])
        hrstd = srows1.tile([1, QW], f32r, tag="rstd")
        nc.scalar.activation(hrstd[:, 0:2], hlnv[:, 0:2], AF.Exp, scale=-0.5)
        pbc = pmm.tile([P, QW], f32, tag="pmm")
        nc.tensor.matmul(pbc[:, 0:2], ones_row[0:1, 0:P], hmean[:, 0:2],
                         start=True, stop=True)
        nc.tensor.matmul(pbc[:, 2:4], ones_row[0:1, 0:P], hrstd[:, 0:2],
                         start=True, stop=True)
        t1 = consts.tile([P, KT, 2], f32, tag="ht1")
        nc.vector.tensor_tensor(
            out=t1[:, :, :], in0=col2, in1=pbc[:, 0:1].to_broadcast([P, KT, 2]),
            op=OP.subtract,
        )
        t2 = consts.tile([P, KT, 2], f32, tag="ht2")
        nc.vector.tensor_tensor(
            out=t2[:, :, :], in0=t1[:, :, :], in1=pbc[:, 2:3].to_broadcast([P, KT, 2]),
            op=OP.mult,
        )
        t3 = consts.tile([P, KT, 2], f32, tag="ht3")
        nc.vector.tensor_tensor(
            out=t3[:, :, :], in0=t2[:, :, :], in1=hls_sb[:, :].to_broadcast([P, KT, 2]),
            op=OP.mult,
        )
        pc = consts.tile([P, KT, 2], f32r, tag="pc")
        nc.vector.tensor_tensor(
            out=pc[:, :, :], in0=t3[:, :, :], in1=hlb_sb[:, :].to_broadcast([P, KT, 2]),
            op=OP.add,
        )
        plog = patt.tile([HD + 1, QW], f32, tag="patt")
        nc.tensor.matmul(
            plog[0:C, 0:2], cb_sb[:, :], ones_row[:, 0:2], start=True, stop=False
        )
        for j in range(KT):
            nc.tensor.matmul(
                plog[0:C, 0:2], cw_sb[:, j, :], pc[:, j, :],
                start=False, stop=(j == KT - 1),
            )
        out_sb = consts.tile([C, 1], f32, tag="outsb")
        nc.vector.tensor_copy(out=out_sb[:, :], in_=plog[0:C, 0:1])
        nc.sync.dma_start(out=d_out[0:1, 0:C], in_=out_sb[0:C, 0:1])

    nc.compile()
    return nc


def _bf16np():
    import ml_dtypes

    return ml_dtypes.bfloat16


def _prep_host(inputs):
    g = lambda k: np.asarray(inputs[k])
    sq = np.float32(math.sqrt(D))
    ids = g("input_ids").astype(np.int32)              # (B, S)
    gm = (1.0 - g("attention_mask").astype(np.float32))  # (B, S)
    emb = (g("token_emb").astype(np.float32) * sq)
    posT = np.ascontiguousarray((g("pos_emb")[:S].astype(np.float32) * sq).T)
    # reference reshapes qkv output to (H, 3, HD): permute columns into
    # contiguous q | k | v blocks (each h-major) before tiling
    idx = np.arange(3 * D).reshape(H, 3, HD)
    cols = np.concatenate(
        [idx[:, 0, :].reshape(-1), idx[:, 1, :].reshape(-1), idx[:, 2, :].reshape(-1)]
    )
    qkv_w = g("qkv_w").astype(np.float32)[:, :, cols].copy()   # (L, D, 3D)
    qkv_b = g("qkv_b").astype(np.float32)[:, cols].copy()      # (L, 3D)
    qkv_w[:, :, :D] *= np.float32(1.0 / math.sqrt(HD))
    qkv_b[:, :D] *= np.float32(1.0 / math.sqrt(HD))
    shared = {
        "emb": emb,
        "posT": posT,
        "qkw": np.ascontiguousarray(
            qkv_w[:, :, : 2 * D].reshape(L, D, 12, P).transpose(0, 2, 1, 3)
        ).astype(_bf16np()),
        "vw": np.ascontiguousarray(qkv_w[:, :, 2 * D :].reshape(L, KT, P, D)).astype(_bf16np()),
        "qkb": np.ascontiguousarray(
            qkv_b[:, : 2 * D].reshape(L, 12, P).transpose(0, 2, 1)
        ),
        "vb": np.ascontiguousarray(qkv_b[:, 2 * D :]),
        "ow": np.ascontiguousarray(
            g("out_w").astype(np.float32).reshape(L, D, KT, P).transpose(0, 2, 1, 3)
        ).astype(_bf16np()),
        "ob": g("out_b").astype(np.float32),
        "n1s": np.ascontiguousarray(
            g("n1_s").astype(np.float32).reshape(L, KT, P).transpose(0, 2, 1)
        ),
        "n1b": np.ascontiguousarray(
            g("n1_b").astype(np.float32).reshape(L, KT, P).transpose(0, 2, 1)
        ),
        "n2s": np.ascontiguousarray(
            g("n2_s").astype(np.float32).reshape(L, KT, P).transpose(0, 2, 1)
        ),
        "n2b": np.ascontiguousarray(
            g("n2_b").astype(np.float32).reshape(L, KT, P).transpose(0, 2, 1)
        ),
        "f1w": np.ascontiguousarray(
            g("ff1_w").astype(np.float32).reshape(L, D, FT, P).transpose(0, 2, 1, 3)
        ).astype(_bf16np()),
        "f1b": np.ascontiguousarray(
            g("ff1_b").astype(np.float32).reshape(L, FT, P).transpose(0, 2, 1)
        ),
        "f2w": np.ascontiguousarray(
            g("ff2_w").astype(np.float32).reshape(L, DF, KT, P).transpose(0, 2, 1, 3)
        ).astype(_bf16np()),
        "f2b": g("ff2_b").astype(np.float32),
        "hls": np.ascontiguousarray(
            g("hln_s").astype(np.float32).reshape(KT, P).T
        ),
        "hlb": np.ascontiguousarray(
            g("hln_b").astype(np.float32).reshape(KT, P).T
        ),
        "cw": g("cls_w").astype(np.float32),
        "cb": g("cls_b").astype(np.float32).reshape(1, C),
    }
    per_core = []
    for c in range(N_CORES):
        per_core.append(
            {
                "ids": np.ascontiguousarray(ids[c].reshape(NT, P).T),
                "gmask": np.ascontiguousarray(gm[c].reshape(NT, P).T),
            }
        )
    return shared, per_core


def _get_nc():
    if "nc" not in _CACHE:
        _CACHE["nc"] = _build_nc()
    return _CACHE["nc"]


def kernel(**inputs):
    from concourse.bass_utils import run_bass_kernel_spmd

    shared, per_core = _prep_host(inputs)
    nc = _get_nc()
    in_maps = [dict(shared, **per_core[c]) for c in range(N_CORES)]
    _CACHE["in_maps"] = in_maps
    res = run_bass_kernel_spmd(nc, in_maps, list(range(N_CORES)))
    out = np.stack([res.results[c]["out"][0] for c in range(N_CORES)], axis=0)
    return out.astype(np.float32)


def bench(n_iters=10):
    """Re-run the compiled NEFF with device-resident inputs; returns the
    best-observed per-iteration wall time in ns (upper bound on HW exec)."""
    import time

    import jax
    import numpy as _np
    from jax.sharding import Mesh, PartitionSpec, NamedSharding
    from jax.experimental.shard_map import shard_map
    from concourse import bass2jax, mybir
    from concourse.bass2jax import _bass_exec_p, install_neuronx_cc_hook

    nc = _get_nc()
    in_maps = _CACHE["in_maps"]
    install_neuronx_cc_hook()

    pname = nc.partition_id_tensor.name if nc.partition_id_tensor else None
    in_names, out_names, out_avals, zero_outs = [], [], [], []
    for alloc in nc.m.functions[0].allocations:
        if not isinstance(alloc, mybir.MemoryLocationSet):
            continue
        name = alloc.memorylocations[0].name
        if alloc.kind == "ExternalInput":
            if name == pname:
                continue
            in_names.append(name)
        elif alloc.kind == "ExternalOutput":
            out_names.append(name)
            shape = tuple(alloc.tensor_shape)
            dtype = mybir.dt.np(alloc.dtype)
            out_avals.append(jax.core.ShapedArray(shape, dtype))
            zero_outs.append(_np.zeros(shape, dtype))
    n_params = len(in_names)
    all_names = in_names + out_names
    if pname is not None:
        all_names = all_names + [pname]

    def _body(*args):
        operands = list(args)
        if pname is not None:
            operands.append(bass2jax.partition_id_tensor())
        outs = _bass_exec_p.bind(
            *operands,
            out_avals=tuple(out_avals),
            in_names=tuple(all_names),
            out_names=tuple(out_names),
            lowering_input_output_aliases=(),
            sim_require_finite=True,
            sim_require_nnan=True,
            nc=nc,
        )
        return tuple(outs)

    devices = jax.devices()[:N_CORES]
    mesh = Mesh(_np.asarray(devices), ("core",))
    nin = n_params + len(zero_outs)
    fn = jax.jit(
        shard_map(
            _body,
            mesh=mesh,
            in_specs=(PartitionSpec("core"),) * nin,
            out_specs=(PartitionSpec("core"),) * len(out_names),
            check_rep=False,
        )
    )
    sharding = NamedSharding(mesh, PartitionSpec("core"))
    concat_in = [
        jax.device_put(
            _np.concatenate([_np.asarray(in_maps[c][n]) for c in range(N_CORES)], 0),
            sharding,
        )
        for n in in_names
    ]
    concat_zeros = [
        jax.device_put(
            _np.zeros((N_CORES * z.shape[0], *z.shape[1:]), z.dtype), sharding
        )
        for z in zero_outs
    ]
    jax.block_until_ready(concat_in)
    # warmup (compile)
    out = fn(*concat_in, *concat_zeros)
    jax.block_until_ready(out)
    # pipelined async dispatch amortizes the axon tunnel round-trip
    outs = []
    t0 = time.perf_counter()
    for _ in range(n_iters):
        outs.append(fn(*concat_in, *concat_zeros))
    jax.block_until_ready(outs)
    dt = (time.perf_counter() - t0) / n_iters
    return int(dt * 1e9)
